# revision 1
# baseline (speedup 1.0000x reference)
"""MoE layer (B=2,S=2048,H=1024,E=8,I=4096,top-2) on 8 Trainium2 NeuronCores.

Expert-parallel sharding: the router (tiny: [4096,1024]@[1024,8] logits,
top-2, softmax) runs on host in float64; tokens are dispatched by router
assignment — core e receives the tokens routed to expert e (gathered,
padded to a common capacity C) plus expert e's weights. Each core runs a
dense SwiGLU FFN in float32r (full-rate PE, ~1e-4 accuracy):

    yT = wd @ ((silu(wg @ xT) * (wu @ xT)))   scaled per-token by the
    top-2 softmax combine weight.

Host scatter-adds the per-expert outputs back into the [T, H] output
(each token appears in exactly 2 experts' batches).

Device layouts (host pre-transposes so every DMA is contiguous):
  xt [8,128,C]      xt[k,p,n] = x_e[n, k*128+p]            (rhs tiles)
  wg,wu [32,128,8,128]  wg[i,p,k,m] = wg_e[i*128+m, k*128+p] (lhsT tiles)
  wd [8,128,32,128]     wd[j,p,i,m] = wd_e[j*128+m, i*128+p]
  ce [128,C]        combine weight broadcast over partitions
  y  [8,128,C]      y[j,p,n] = out_e[n, j*128+p]

The kernel runs in 2 passes of 16 I-chunks each so that the hidden
tiles (f32r [128, C] x 16) fit in SBUF alongside xt and the fp32 y
accumulator; wd partials from the two passes are summed in SBUF.
"""
import sys

import numpy as np

for _p in ("/opt/trn_rl_repo", "/root/.axon_site/_ro/trn_rl_repo"):
    if _p not in sys.path:
        sys.path.append(_p)

import concourse.bacc as bacc
import concourse.mybir as mybir
import concourse.tile as tile
from concourse import bass_utils

B, S, H, E, I, K = 2, 2048, 1024, 8, 4096, 2
T = B * S
KH = H // 128      # 8 contraction chunks for H
NI = I // 128      # 32 I-chunks
NJ = H // 128      # 8 output H-chunks
def _npass_for(C: int) -> int:
    """More passes shrink the resident hidden-state tiles (IPP*4*C B per
    partition) at no PE cost; pick the fewest passes that fit SBUF."""
    for npass in (2, 4, 8):
        ipp = NI // npass
        if (76 + 4 * ipp) * C + 40 * 1024 < 200 * 1024:
            return npass
    return 8

F32 = mybir.dt.float32
F32R = mybir.dt.float32r
SILU = mybir.ActivationFunctionType.Silu

_module_cache = {}


def _chunk_plan(n_max: int):
    """Uniform token chunks: <=512 (PSUM bank), >=256 (f32r full rate),
    multiple of 8 (ISA constraint on f32r matmul free dim)."""
    n = max(int(n_max), 256)
    nchunks = -(-n // 512)
    base = min(512, max(256, -(-(-(-n // nchunks)) // 8) * 8))
    return (base,) * nchunks


def build_module(chunks, reps: int = 1, act=SILU, bufs=None, loop_reps: int = 0):
    bufs = {**{"wg": 2, "wu": 2, "wd": 2, "sil": 3, "yo": 2,
               "psg": 2, "psu": 2, "psy": 2}, **(bufs or {})}
    """Build + compile the per-core Bass module. `reps` unrolls the whole
    body multiple times (timing amplification only; outputs identical)."""
    C = sum(chunks)
    NPASS = _npass_for(C)
    IPP = NI // NPASS
    offs = np.cumsum([0] + list(chunks))[:-1]
    nc = bacc.Bacc("TRN2", target_bir_lowering=False, debug=False)

    xt_d = nc.dram_tensor("xt", [len(chunks), KH, 128, max(chunks)], F32R,
                          kind="ExternalInput")
    wg_d = nc.dram_tensor("wg", [NI, 128, KH, 128], F32R, kind="ExternalInput")
    wu_d = nc.dram_tensor("wu", [NI, 128, KH, 128], F32R, kind="ExternalInput")
    wd_d = nc.dram_tensor("wd", [NJ, 128, NI, 128], F32R, kind="ExternalInput")
    ce_d = nc.dram_tensor("ce", [128, C], F32, kind="ExternalInput")
    y_d = nc.dram_tensor("y", [NJ, 128, C], F32, kind="ExternalOutput")

    with tile.TileContext(nc) as tc:
        with (
            tc.tile_pool(name="xp", bufs=1) as xp,
            tc.tile_pool(name="cep", bufs=1) as cep,
            tc.tile_pool(name="yacc", bufs=1) as yaccp,
            tc.tile_pool(name="hp", bufs=1) as hp,
            tc.tile_pool(name="wgp", bufs=bufs["wg"]) as wgp,
            tc.tile_pool(name="wup", bufs=bufs["wu"]) as wup,
            tc.tile_pool(name="wdp", bufs=bufs["wd"]) as wdp,
            tc.tile_pool(name="silp", bufs=bufs["sil"]) as silp,
            tc.tile_pool(name="yop", bufs=bufs["yo"]) as yop,
            tc.tile_pool(name="psg", bufs=bufs["psg"], space="PSUM") as psg,
            tc.tile_pool(name="psu", bufs=bufs["psu"], space="PSUM") as psu,
            tc.tile_pool(name="psy", bufs=bufs["psy"], space="PSUM") as psy,
        ):
            # xt loads ride the gpsimd SWDGE queue so the first weight tiles
            # (sync queue) issue in parallel; chunk 0 loads first so the
            # first matmuls start as early as possible.
            xts = [xp.tile([128, C], F32R, tag=f"x{k}", name=f"xt{k}")
                   for k in range(KH)]
            for c, (off, nck) in enumerate(zip(offs, chunks)):
                for k in range(KH):
                    nc.gpsimd.dma_start(xts[k][:, off:off + nck],
                                        xt_d[c][k][:, :nck])
            cet = cep.tile([128, C], F32)
            nc.gpsimd.dma_start(cet[:], ce_d[:])
            yacc = [yaccp.tile([128, C], F32, tag=f"y{j}", name=f"yacc{j}")
                    for j in range(NJ)]

            def stage1_weights(i, fine=False):
                wgt = wgp.tile([128, KH, 128], F32R, tag="wg", name=f"wg_{i}")
                wut = wup.tile([128, KH, 128], F32R, tag="wu", name=f"wu_{i}")
                # split loads so the first matmul waits on a fraction of the
                # 512KB tile: per-k slices (64KB) for the head iterations,
                # halves elsewhere
                step = 1 if fine else KH // 2
                for a in range(0, KH, step):
                    nc.sync.dma_start(wgt[:, a:a + step, :],
                                      wg_d[i][:, a:a + step, :])
                for a in range(0, KH, step):
                    nc.sync.dma_start(wut[:, a:a + step, :],
                                      wu_d[i][:, a:a + step, :])
                return wgt, wut

            def stage1_chunk(h_tiles, il, c, wgt, wut):
                off, nck = offs[c], chunks[c]
                pg = psg.tile([128, nck], F32, tag="pg", name=f"pg_{il}_{c}")
                pu = psu.tile([128, nck], F32, tag="pu", name=f"pu_{il}_{c}")
                for k in range(KH):
                    nc.tensor.matmul(
                        pg[:], wgt[:, k, :], xts[k][:, off:off + nck],
                        start=(k == 0), stop=(k == KH - 1),
                    )
                for k in range(KH):
                    nc.tensor.matmul(
                        pu[:], wut[:, k, :], xts[k][:, off:off + nck],
                        start=(k == 0), stop=(k == KH - 1),
                    )
                sl = silp.tile([128, nck], F32, tag="sil", name=f"sl_{il}_{c}")
                nc.scalar.activation(sl[:], pg[:], act)
                h = hp.tile([128, nck], F32R, tag=f"h{il}_{c}", name=f"h_{il}_{c}")
                nc.vector.tensor_mul(h[:], sl[:], pu[:])
                h_tiles[(il, c)] = h

            def body():
                for p in range(NPASS):
                    h_tiles = {}
                    if p == 0:
                        # skewed start: run the first two I-chunks
                        # column-by-column so chunks 1/2 of xt aren't
                        # needed until their DMAs have landed
                        w0 = stage1_weights(0)
                        w1 = stage1_weights(1)
                        for c in range(len(chunks)):
                            stage1_chunk(h_tiles, 0, c, *w0)
                            stage1_chunk(h_tiles, 1, c, *w1)
                        start_il = 2
                    else:
                        start_il = 0
                    for il in range(start_il, IPP):
                        i = p * IPP + il
                        wgt, wut = stage1_weights(i)
                        for c in range(len(chunks)):
                            stage1_chunk(h_tiles, il, c, wgt, wut)
                    for j in range(NJ):
                        wdt = wdp.tile([128, IPP, 128], F32R, tag="wd")
                        nc.sync.dma_start(
                            wdt[:], wd_d[j][:, p * IPP:(p + 1) * IPP, :])
                        for c, (off, nck) in enumerate(zip(offs, chunks)):
                            py = psy.tile([128, nck], F32)
                            for il in range(IPP):
                                nc.tensor.matmul(
                                    py[:], wdt[:, il, :], h_tiles[(il, c)][:],
                                    start=(il == 0), stop=(il == IPP - 1),
                                )
                            ya = yacc[j][:, off:off + nck]
                            if p == 0:
                                nc.vector.tensor_copy(ya, py[:])
                            else:
                                nc.vector.tensor_add(ya, ya, py[:])
                            if p == NPASS - 1:
                                # fused combine-scale + store per slice so the
                                # kernel tail is one slice, not a full pass
                                yo = yop.tile([128, nck], F32, tag="yo")
                                nc.vector.tensor_mul(
                                    yo[:], ya, cet[:, off:off + nck])
                                nc.sync.dma_start(y_d[j][:, off:off + nck],
                                                  yo[:])

            if loop_reps > 0:
                with tc.For_i(0, loop_reps, 1):
                    body()
            else:
                for _rep in range(reps):
                    body()
    nc.compile()
    return nc


def _get_module(chunks, reps=1):
    key = (chunks, reps)
    if key not in _module_cache:
        _module_cache[key] = build_module(chunks, reps)
    return _module_cache[key]


def _route(x_flat: np.ndarray, gate_w: np.ndarray):
    """Router replicating reference bit-for-bit: fp32 logits, top-2,
    softmax — via jax on the default backend (same code path the
    reference takes), so expert selection matches the graded reference
    even for near-tied logits. Falls back to numpy fp32/fp64 if jax is
    unusable."""
    try:
        import jax
        import jax.numpy as jnp

        logits = jnp.asarray(x_flat) @ jnp.asarray(gate_w).T
        top_v, top_i = jax.lax.top_k(logits, K)
        probs = jax.nn.softmax(top_v.astype(jnp.float32), axis=-1)
        top_i = np.asarray(top_i)
        probs = np.asarray(probs, dtype=np.float32)
        return top_i[:, 0], top_i[:, 1], probs[:, 0], probs[:, 1]
    except Exception:
        logits = x_flat.astype(np.float64) @ gate_w.astype(np.float64).T
        order = np.argsort(-logits, axis=1)
        i1, i2 = order[:, 0], order[:, 1]
        rows = np.arange(logits.shape[0])
        p1 = 1.0 / (1.0 + np.exp(logits[rows, i2] - logits[rows, i1]))
        return i1, i2, p1.astype(np.float32), (1.0 - p1).astype(np.float32)


def make_in_maps(x_flat, gate_w, wg, wu, wd, chunks=None):
    """Returns (in_maps, idx_list, n_list, chunks)."""
    i1, i2, p1, p2 = _route(x_flat, gate_w)
    tok = np.concatenate([np.arange(T), np.arange(T)])
    exp = np.concatenate([i1, i2])
    prob = np.concatenate([p1, p2])
    idx_list, prob_list = [], []
    for e in range(E):
        m = exp == e
        idx_list.append(tok[m])
        prob_list.append(prob[m])
    n_list = [len(ix) for ix in idx_list]
    if chunks is None:
        chunks = _chunk_plan(max(n_list))
    C = sum(chunks)

    in_maps = []
    for e in range(E):
        ix, n_e = idx_list[e], n_list[e]
        xe = np.zeros((C, H), np.float32)
        xe[:n_e] = x_flat[ix]
        xeT = xe.T  # [H, C]
        mx = max(chunks)
        xt = np.zeros((len(chunks), KH, 128, mx), np.float32)
        for c, (off, nck) in enumerate(
                zip(np.cumsum([0] + list(chunks))[:-1], chunks)):
            xt[c, :, :, :nck] = xeT[:, off:off + nck].reshape(KH, 128, nck)
        wgd = np.ascontiguousarray(
            wg[e].T.reshape(KH, 128, NI, 128).transpose(2, 1, 0, 3))
        wud = np.ascontiguousarray(
            wu[e].T.reshape(KH, 128, NI, 128).transpose(2, 1, 0, 3))
        wdd = np.ascontiguousarray(
            wd[e].T.reshape(NI, 128, NJ, 128).transpose(2, 1, 0, 3))
        ceb = np.zeros((128, C), np.float32)
        ceb[:, :n_e] = prob_list[e]
        in_maps.append({"xt": xt, "wg": wgd, "wu": wud, "wd": wdd, "ce": ceb})
    return in_maps, idx_list, n_list, chunks


def combine_outputs(results, idx_list, n_list, C):
    out = np.zeros((T, H), np.float32)
    for e in range(E):
        yT = results[e]["y"].reshape(H, C)
        out[idx_list[e]] += yT[:, :n_list[e]].T
    return out.reshape(B, S, H)


def _run_with_retry(nc, in_maps, attempts=3):
    """The axon terminal takes a while to accept a new session right after
    the previous client disconnected; a too-early execute surfaces as
    'accelerator device unrecoverable'. Clear jax backends, wait, retry."""
    import time

    for a in range(attempts):
        try:
            return bass_utils.run_bass_kernel_spmd(
                nc, in_maps, core_ids=list(range(E)))
        except Exception:
            if a == attempts - 1:
                raise
            try:
                import jax

                jax.clear_caches()
                jax.extend.backend.clear_backends()
            except Exception:
                pass
            time.sleep(30 * (a + 1))


def kernel(x, gate_w, wg, wu, wd):
    x = np.asarray(x, np.float32)
    gate_w = np.asarray(gate_w, np.float32)
    wg = np.asarray(wg, np.float32)
    wu = np.asarray(wu, np.float32)
    wd = np.asarray(wd, np.float32)
    x_flat = x.reshape(T, H)

    in_maps, idx_list, n_list, chunks = make_in_maps(x_flat, gate_w, wg, wu, wd)
    nc = _get_module(chunks)
    res = _run_with_retry(nc, in_maps)
    return combine_outputs(res.results, idx_list, n_list, sum(chunks))

